# revision 23
# baseline (speedup 1.0000x reference)
"""Trainium2 Bass kernel for nn_MultiHeadCrossAttention (B,C,H,W = 8,512,64,64).

Self-contained: builds one single-core Bass/Tile program and runs it SPMD on
8 NeuronCores (data-parallel, one batch element per core).

v2: fully pipelined by 512-token chunk.  x is DMA'd in (t8, i) chunks on the
sync queue (weights go on the scalar queue so they never stall the x stream);
G folds, V projection, SK, scores/exp all chase the DMA stream.  V is stored
dd-contiguous so the AV stage's LDWEIGHTS are contiguous, and AV matmuls are
interleaved over (r2, n2) PE quadrants so 64x64 matmuls overlap in the array.
The PE instruction stream is kept dense so the HAM clock gate stays at 2.4GHz.
"""
import sys

for _p in ("/opt/trn_rl_repo", "/root/.axon_site/_ro/trn_rl_repo"):
    if _p not in sys.path:
        sys.path.append(_p)

import numpy as np


# ---------------------------------------------------------------------------
# Workaround: this walrus build caps sync-waits per CTRL instruction; the
# TileContext exit drain accumulates one wait per active processor and blows
# the cap.  Pre-absorb each wait on its own SP nop before the drain.
# ---------------------------------------------------------------------------
def _install_drain_patch():
    import concourse.tile as tile
    from concourse.vector_clock import ScopedClock

    if getattr(tile.TileContext, "_drain_patch_installed", False):
        return

    def _patched(self, tick_clock, wait_clock):
        nc = self.nc
        gc = tick_clock.global_clock
        scoped = gc if hasattr(gc, "items") else ScopedClock({None: gc})
        for scope, clock in scoped.items():
            for i in range(32):
                try:
                    t = clock.peek_next(i) - 1
                except Exception:
                    break
                if t > 0:
                    nop = nc.sync.nop(nofuse=True, hint="drain_split")
                    sc = ScopedClock()
                    sc.require_at_least(scope, i, t)
                    wait_clock.add_sem_waits(nop.ins, sc)
        nc.sync.drain()  # nops above absorbed every wait; SP is in-order

        nc.all_engine_barrier()
        assert self.sems is not None
        popped = nc._tile_sem_poison_stack.pop()
        assert popped is self._sem_poison
        nc.clear_and_free_semaphores(list(self.sems.allocated().values()))
        nc.all_engine_barrier()

    tile.TileContext._drain_and_barrier = _patched
    tile.TileContext._drain_patch_installed = True


import concourse.bass as bass
import concourse.tile as tile
from concourse import mybir

F32 = mybir.dt.float32
F32R = mybir.dt.float32r
BF16 = mybir.dt.bfloat16
AF = mybir.ActivationFunctionType
ALU = mybir.AluOpType
AX = mybir.AxisListType

C, HW, NH, D, H, W = 512, 4096, 8, 64, 64, 64
NB = 4  # 128-partition blocks of C


def _split_excess_waits(nc, cap=2):
    """This walrus build caps sync-waits per ISA instruction.  Move excess
    waits onto same-engine NoOps inserted just before the instruction
    (same engine => executes immediately before it; semantically identical)."""
    k = 0
    for fn in nc.m.functions:
        for blk in fn.blocks:
            out, changed = [], False
            for inst in blk.instructions:
                si = inst.sync_info
                icap = 1
                if si is not None and len(si.on_wait) > icap:
                    waits = list(si.on_wait)
                    excess, keep = waits[:-icap], waits[-icap:]
                    while excess:
                        chunk, excess = excess[:1], excess[1:]
                        k += 1
                        nop = mybir.InstNoOp(
                            name=f"I-waitsplit-{k}", engine=inst.engine
                        )
                        nop.sync_info = mybir.SyncInfo(
                            on_wait=chunk, on_update=[]
                        )
                        nc.register_instruction(nop)
                        out.append(nop)
                    inst.sync_info = mybir.SyncInfo(
                        on_wait=keep, on_update=list(si.on_update)
                    )
                    changed = True
                out.append(inst)
            if changed:
                blk.instructions = out
    return k


def build():
    nc = bass.Bass("TRN2", target_bir_lowering=False, debug=False, num_devices=1)

    hidden = nc.dram_tensor("hidden", [C, HW], F32R, kind="ExternalInput")
    guide = nc.dram_tensor("guide", [1, C], F32, kind="ExternalInput")
    Wq = nc.dram_tensor("Wq", [C, C], F32, kind="ExternalInput")
    Wk = nc.dram_tensor("Wk", [C, C], F32, kind="ExternalInput")
    Wv = nc.dram_tensor("Wv", [C, C], F32, kind="ExternalInput")
    bq = nc.dram_tensor("bq", [1, C], F32, kind="ExternalInput")
    bk = nc.dram_tensor("bk", [1, C], F32, kind="ExternalInput")
    bv = nc.dram_tensor("bv", [1, C], F32, kind="ExternalInput")
    out = nc.dram_tensor("out", [C, HW], F32, kind="ExternalOutput")

    with tile.TileContext(nc) as tc:
        _body(nc, tc, hidden, guide, Wq, Wk, Wv, bq, bk, bv, out)
    _split_excess_waits(nc)
    return nc


def _body(nc, tc, hidden, guide, Wq, Wk, Wv, bq, bk, bv, out):
    import contextlib

    ctx = contextlib.ExitStack()
    with ctx:
        P = ctx.enter_context(tc.tile_pool(name="persist", bufs=1))
        WN = ctx.enter_context(tc.tile_pool(name="wnat", bufs=4))
        GF = ctx.enter_context(tc.tile_pool(name="gfold", bufs=1))
        SC = ctx.enter_context(tc.tile_pool(name="scpool", bufs=2))
        SS = ctx.enter_context(tc.tile_pool(name="sksb", bufs=1))
        WQ = ctx.enter_context(tc.tile_pool(name="wtq", bufs=1))
        AS = ctx.enter_context(tc.tile_pool(name="attsb", bufs=2))
        PS = ctx.enter_context(tc.tile_pool(name="ps", bufs=2, space="PSUM"))
        PZ = ctx.enter_context(tc.tile_pool(name="pz", bufs=2, space="PSUM"))
        PA = ctx.enter_context(tc.tile_pool(name="pa", bufs=4, space="PSUM"))

        # ---------------- constants ----------------
        ident = P.tile([128, 128], F32, tag="ident")
        from concourse.masks import make_identity

        make_identity(nc, ident[:])
        one1 = P.tile([1, 1], F32, tag="one1")
        nc.vector.memset(one1[:], 1.0)
        # ones column with entries at bases 0/32/64/96 (identity for row
        # transposes of the packed guide/bias rows)
        one_pk = P.tile([97, 1], F32, tag="one_pk")
        nc.vector.memset(one_pk[:], 1.0)
        identpk = P.tile([66, 2], F32, tag="identpk")
        nc.vector.memset(identpk[0:2, :], 0.0)
        # half-indicator rows for QV2 broadcast (K=1 matmuls)
        ones1a = P.tile([1, 128], F32, tag="ones1a")
        ones1b = P.tile([1, 128], F32, tag="ones1b")
        nc.vector.memset(ones1a[:], 0.0)
        nc.vector.memset(ones1a[:, 0:64], 1.0)
        nc.vector.memset(ones1b[:], 0.0)
        nc.vector.memset(ones1b[:, 64:128], 1.0)
        # column indicator [128,2] (lhsT for Z row sums over each 64-half)
        i2colsT = P.tile([128, 2], BF16, tag="i2colsT")
        nc.vector.memset(i2colsT[:], 0.0)
        nc.vector.memset(i2colsT[0:64, 0:1], 1.0)
        nc.vector.memset(i2colsT[64:128, 1:2], 1.0)
        # [2,128] row-indicator blocks at partition bases 32*j2 (lhsT for the
        # partition-broadcast of zinv rows; base must match the zn rhs base)
        i2colsF = P.tile([128, 2], F32, tag="i2colsF")
        nc.vector.memset(i2colsF[:], 0.0)
        nc.vector.memset(i2colsF[0:64, 0:1], 1.0)
        nc.vector.memset(i2colsF[64:128, 1:2], 1.0)
        i2rowsB = P.tile([98, 128], BF16, tag="i2rowsB")
        # ~5us of dummy matmuls: trips the HAM activity monitor so the PE
        # clock is at 2.4GHz before the V projection starts (it would
        # otherwise run the whole x-stream phase at 1.2GHz)
        warm = PZ.tile([128, 128], F32, tag="small")
        for _ in range(60):
            nc.tensor.matmul(out=warm[:], lhsT=ident[:], rhs=ident[:],
                             start=True, stop=True)
        i2rp = PZ.tile([2, 128], F32, tag="small")
        nc.tensor.transpose(out=i2rp[:], in_=i2colsF[:], identity=ident[:])
        for j2 in range(NB):
            nc.vector.tensor_copy(i2rowsB[32 * j2:32 * j2 + 2, :], i2rp[:])

        # ------- input DMAs: weights on scalar queue, x on sync queue -------
        wnat = {}
        for nm, dr in (("Wv", Wv), ("Wk", Wk), ("Wq", Wq)):
            nat = []
            for cb in range(NB):
                t = WN.tile([128, C], F32, tag="wtrans")
                nc.scalar.dma_start(out=t, in_=dr.ap()[cb * 128:(cb + 1) * 128, :])
                nat.append(t)
            wnat[nm] = nat
        # guide + biases packed at partition bases 0/32/64/96 of one tile
        rowpk = P.tile([97, C], F32, tag="rowpk")
        nc.scalar.dma_start(out=rowpk[0:1, :], in_=guide.ap())
        nc.scalar.dma_start(out=rowpk[32:33, :], in_=bq.ap())
        nc.scalar.dma_start(out=rowpk[64:65, :], in_=bk.ap())
        nc.scalar.dma_start(out=rowpk[65:66, :], in_=bv.ap())
        nc.sync.dma_start(out=identpk[64:66, :], in_=ident[0:2, 0:2])
        guide_sb = rowpk[0:1, :]
        brow = {"bq": rowpk[32:33, :], "bkv": rowpk[64:66, :]}
        # x tiles, DMA'd per (t8, i) 512-token chunk so compute can chase
        x = [P.tile([128, HW], F32R, tag=f"x{j}", name=f"x{j}") for j in range(NB)]
        for t8 in range(8):
            for i in range(NB):
                nc.sync.dma_start(
                    out=x[i][:, t8 * 512:(t8 + 1) * 512],
                    in_=hidden.ap()[i * 128:(i + 1) * 128,
                                    t8 * 512:(t8 + 1) * 512],
                )

        # ---------------- weight transposes (PE) ----------------
        # wT[j] [128,512]: wT[j][p, c_out] = W[c_out, 128j+p]
        def transpose_weight(nm, j, dst_pool, dtype, copy_eng, tag=None):
            pt = PS.tile([128, C], F32, tag="mm512")
            for cb in range(NB):
                nc.tensor.transpose(
                    out=pt[:, cb * 128:(cb + 1) * 128],
                    in_=wnat[nm][cb][:, j * 128:(j + 1) * 128],
                    identity=ident[:],
                )
            sb = dst_pool.tile([128, C], dtype, tag=tag or f"{nm}T{j}",
                               name=f"{nm}T{j}")
            if copy_eng is nc.scalar:
                copy_eng.copy(sb[:], pt[:])
            else:
                copy_eng.tensor_copy(sb[:], pt[:])
            return sb

        wTv = [transpose_weight("Wv", j, P, F32R, nc.vector) for j in range(NB)]  # noqa
        wTk = [transpose_weight("Wk", j, P, F32, nc.scalar) for j in range(NB)]

        # ---------------- bias columns (PE transpose of rows) ----------------
        bkcol, bvcol = [], []
        for j in range(NB):
            pt = PZ.tile([128, 2], F32, tag="small")
            nc.tensor.transpose(
                out=pt[:], in_=brow["bkv"][:, j * 128:(j + 1) * 128],
                identity=identpk[64:66, :], tile_position=(64, 0),
            )
            kc = P.tile([128, 1], F32, tag=f"bk64c{j}")
            vc = P.tile([128, 1], F32, tag=f"bvc{j}")
            nc.scalar.mul(kc[:], pt[:, 0:1], 64.0)  # K bias enters via 64-tok sum
            nc.scalar.copy(vc[:], pt[:, 1:2])
            bkcol.append(kc)
            bvcol.append(vc)
        gcol = []
        for j in range(NB):
            pt = PZ.tile([128, 1], F32, tag="small")
            nc.tensor.transpose(
                out=pt[:], in_=guide_sb[:, j * 128:(j + 1) * 128],
                identity=one1[:],
            )
            t = P.tile([128, 1], F32, tag=f"gcol{j}")
            nc.vector.tensor_copy(t[:], pt[:])
            gcol.append(t)

        # ---------------- q path (per-block WqT, consumed immediately) -----
        qp = PZ.tile([1, C], F32, tag="small")
        for i in range(NB):
            wTq_i = transpose_weight("Wq", i, WQ, F32, nc.scalar, tag="wtq")
            nc.tensor.matmul(
                out=qp[:], lhsT=gcol[i][:], rhs=wTq_i[:],
                start=(i == 0), stop=False,
            )
        nc.tensor.matmul(
            out=qp[:], lhsT=one_pk[32:33, 0:1], rhs=brow["bq"],
            start=False, stop=True, tile_position=(32, 0),
        )
        q_sc = P.tile([1, C], F32, tag="q_sc")
        nc.scalar.copy(q_sc[:], qp[:])
        qv2 = []
        for j in range(NB):
            pt = PZ.tile([128, 64], F32, tag="small")
            nc.tensor.matmul(
                out=pt[:], lhsT=ones1a[:],
                rhs=q_sc[:, (2 * j) * 64:(2 * j + 1) * 64],
                start=True, stop=False,
            )
            nc.tensor.matmul(
                out=pt[:], lhsT=ones1b[:],
                rhs=q_sc[:, (2 * j + 1) * 64:(2 * j + 2) * 64],
                start=False, stop=True,
            )
            t = P.tile([128, 64], F32, tag=f"qv2_{j}")
            nc.scalar.mul(t[:], pt[:], 0.125)  # fold scale = 1/sqrt(d)
            qv2.append(t)

        # ---------------- persistent state for the chunk pipeline ----------
        g = [P.tile([128, 64], F32, tag=f"g{j}", name=f"g{j}") for j in range(NB)]
        vt = [P.tile([128, HW], BF16, tag=f"vt{j}", name=f"vt{j}") for j in range(NB)]
        e = [P.tile([128, HW], BF16, tag=f"e{j2}", name=f"e{j2}") for j2 in range(NB)]
        skT = [P.tile([128, 64], F32, tag=f"skT{j2}", name=f"skT{j2}") for j2 in range(NB)]
        # Z staging packed at partition bases 32*j2 (rows 32*j2 + r2); the
        # unused rows make the free-range shared so SBUF cost is one tile.
        zsbig = P.tile([98, HW], BF16, tag="zsbig")
        znbig = P.tile([98, HW], BF16, tag="znbig")
        # zr[j2] [128, 64]: p = r2*64 + (n*8 + g'), f = ww
        zr = [P.tile([128, 64], BF16, tag=f"zr{j2}", name=f"zr{j2}") for j2 in range(NB)]
        zinv = [P.tile([128, 64], BF16, tag=f"zi{j2}", name=f"zinv{j2}") for j2 in range(NB)]

        def g_chunk(i, t8):
            # fold dd 64->32 on gpsimd, then DVE strided reduce over dd(32);
            # two ops per chunk so G tracks the x DMA stream with ~1us lag
            xc = x[i][:, t8 * 512:(t8 + 1) * 512].bitcast(F32)
            fa = GF.tile([128, 256], F32, tag="fa", name=f"fa{i}_{t8}")
            nc.gpsimd.tensor_add(fa[:], xc[:, 0:256], xc[:, 256:512])
            nc.vector.tensor_reduce(
                out=g[i][:, t8 * 8:(t8 + 1) * 8],
                in_=fa[:].rearrange("p (dd gg) -> p gg dd", gg=8),
                axis=AX.X, op=ALU.add,
            )

        def vproj_chunk(t8, j, eng):
            pt = PS.tile([128, 512], F32, tag="mm512")
            for i in range(NB):
                nc.tensor.matmul(
                    out=pt[:],
                    lhsT=wTv[i][:, j * 128:(j + 1) * 128],
                    rhs=x[i][:, t8 * 512:(t8 + 1) * 512],
                    start=(i == 0), stop=(i == NB - 1),
                )
            # permute free axis (dd*8+g') -> (g'*64+dd) so AV lhsT is contiguous
            dst = vt[j][:, t8 * 512:(t8 + 1) * 512].rearrange(
                "p (gg dd) -> p gg dd", gg=8)
            src = pt[:].rearrange("p (dd gg) -> p gg dd", gg=8)
            if eng % 2 == 0:
                nc.scalar.activation(dst, src, AF.Identity, bias=bvcol[j][:])
            else:
                nc.vector.tensor_scalar_add(dst, src, bvcol[j][:])

        def sk_half(h):
            # SKT2h [32, 512] = (G-half)^T contracted with WkT, then PE
            # transpose per j2-block into skT[j2][:, h*32:(h+1)*32]
            spt = PZ.tile([32, 512], F32, tag="small")
            for i in range(NB):
                nc.tensor.matmul(
                    out=spt[:], lhsT=g[i][:, h * 32:(h + 1) * 32],
                    rhs=wTk[i][:], start=(i == 0), stop=(i == NB - 1),
                )
            ssb = SS.tile([32, 512], F32, tag="sksb")
            nc.scalar.copy(ssb[:], spt[:])
            for j2 in range(NB):
                tpt = PZ.tile([128, 32], F32, tag="small")
                nc.tensor.transpose(
                    out=tpt[:], in_=ssb[:, j2 * 128:(j2 + 1) * 128],
                    identity=ident[0:32, 0:32],
                )
                nc.vector.tensor_scalar_add(
                    skT[j2][:, h * 32:(h + 1) * 32], tpt[:], bkcol[j2][:])

        def scores_exp_chunk(j2, n):
            sc_t = SC.tile([128, 512], F32, tag="sc")
            in0 = (qv2[j2][:].rearrange("p (a q) -> p a q", a=1)
                   .broadcast_to((128, 8, 64)))
            in1 = skT[j2][:, n * 8:(n + 1) * 8].broadcast_to((128, 8, 64))
            o3 = sc_t[:].rearrange("p (s q) -> p s q", q=64)
            if n < 4:
                nc.gpsimd.tensor_mul(o3, in0, in1)
            else:
                nc.vector.tensor_mul(o3, in0, in1)
            nc.scalar.activation(
                e[j2][:, n * 512:(n + 1) * 512], sc_t[:], AF.Exp)

        # ---------------- chunk pipeline ----------------
        def z_group(n):
            zpb = PZ.tile([98, 512], F32, tag="small", name=f"zpb{n}")
            nc.vector.memset(zpb[:], 0.0)
            for j2 in range(NB):
                nc.tensor.matmul(
                    out=zpb[32 * j2:32 * j2 + 2, :], lhsT=i2colsT[:],
                    rhs=e[j2][:, n * 512:(n + 1) * 512],
                    start=True, stop=True, tile_position=(0, 32 * j2),
                )
            with nc.allow_low_precision(reason="bf16 Z staging"):
                if n % 2 == 0:
                    nc.vector.tensor_copy(zsbig[:, n * 512:(n + 1) * 512], zpb[:])
                else:
                    nc.scalar.copy(zsbig[:, n * 512:(n + 1) * 512], zpb[:])

        def z_tail_half(h):
            # gather Z rows for this n-half into 32-aligned partition blocks
            # of zr, reciprocal, scatter back to znbig.  DMAs ride the vector
            # ring so they never queue behind 1MB output DMAs.
            for j2 in range(NB):
                for r2 in range(2):
                    nc.sync.dma_start(
                        out=zr[j2][r2 * 64 + h * 32:r2 * 64 + (h + 1) * 32, :],
                        in_=zsbig[32 * j2 + r2:32 * j2 + r2 + 1,
                                  h * 2048:(h + 1) * 2048].rearrange(
                            "p (a w) -> p a w", w=64),
                    )
                with nc.allow_low_precision(reason="bf16 softmax normalizer"):
                    for r2 in range(2):
                        sl = slice(r2 * 64 + h * 32, r2 * 64 + (h + 1) * 32)
                        nc.vector.reciprocal(zinv[j2][sl, :], zr[j2][sl, :])
                for r2 in range(2):
                    nc.sync.dma_start(
                        out=znbig[32 * j2 + r2:32 * j2 + r2 + 1,
                                  h * 2048:(h + 1) * 2048].rearrange(
                            "p (a w) -> p a w", w=64),
                        in_=zinv[j2][r2 * 64 + h * 32:r2 * 64 + (h + 1) * 32, :],
                    )

        def norm_half(j2, h):
            for n in range(h * 4, (h + 1) * 4):
                zb = PZ.tile([128, 512], F32, tag="small")
                nc.tensor.matmul(
                    out=zb[:], lhsT=i2rowsB[32 * j2:32 * j2 + 2, :],
                    rhs=znbig[32 * j2:32 * j2 + 2, n * 512:(n + 1) * 512],
                    start=True, stop=True, tile_position=(32 * j2, 0),
                )
                es = e[j2][:, n * 512:(n + 1) * 512]
                nc.vector.tensor_mul(es, es, zb[:])

        for t8 in range(8):
            if t8 < 7:
                for i in range(NB):
                    g_chunk(i, t8)
            if t8 == 6:
                for i in range(NB):
                    g_chunk(i, 7)
            for j in range(NB):
                vproj_chunk(t8, j, eng=(t8 * NB + j) % 2)
            if t8 == 3:
                sk_half(0)
                for n in range(4):
                    for j2 in range(NB):
                        scores_exp_chunk(j2, n)
            if t8 == 6:
                sk_half(1)
                for n in range(4):
                    z_group(n)
                for n in range(4, 8):
                    for j2 in range(NB):
                        scores_exp_chunk(j2, n)

        z_tail_half(0)

        def av_block(j2, jp):
            if True:
                at = [PA.tile([128, 512], F32, tag="att", name=f"at{j2}_{jp}_{k}")
                      for k in range(2)]
                for gg in range(8):
                    for r2, n2 in ((0, 0), (1, 1), (0, 1), (1, 0)):
                        n = 2 * jp + n2
                        sl = slice(n * 512 + gg * 64, n * 512 + (gg + 1) * 64)
                        nc.tensor.matmul(
                            out=at[r2][n2 * 64:(n2 + 1) * 64,
                                       gg * 64:(gg + 1) * 64],
                            lhsT=vt[j2][r2 * 64:(r2 + 1) * 64, sl],
                            rhs=e[j2][r2 * 64:(r2 + 1) * 64, sl],
                            start=True, stop=True,
                        )
                asb = AS.tile([128, 1024], F32, tag="attsb",
                              name=f"asb{j2}_{jp}")
                av = asb[:].rearrange("p (gg rr q) -> p gg rr q", rr=2, q=64)
                for r2 in range(2):
                    srcv = at[r2][:].rearrange("p (gg q) -> p gg q", q=64)
                    if (jp + r2) % 2 == 0:
                        nc.vector.tensor_copy(av[:, :, r2, :], srcv)
                    else:
                        nc.scalar.copy(av[:, :, r2, :], srcv)
                dst = out.ap()[jp * 128:(jp + 1) * 128, :].rearrange(
                    "p (gg rr2 q2) -> p rr2 gg q2", rr2=4, q2=128
                )[:, j2, :, :]
                srcd = asb[:].rearrange("p (gg q2) -> p gg q2", q2=128)
                eng = [nc.sync, nc.scalar][(j2 + jp) % 2]
                eng.dma_start(out=dst, in_=srcd)

        # half 0 (jp 0-1) normalizes and runs right after Vproj; the half-1
        # Z chain is interleaved mid-stream so the PE never idles.
        for j2 in range(NB):
            norm_half(j2, 0)
        av_block(0, 0)
        av_block(0, 1)
        for n in range(4, NH):
            z_group(n)
        av_block(1, 0)
        av_block(1, 1)
        z_tail_half(1)
        av_block(2, 0)
        av_block(2, 1)
        for j2 in range(NB):
            norm_half(j2, 1)
        av_block(3, 0)
        av_block(3, 1)
        for j2 in range(NB):
            av_block(j2, 2)
            av_block(j2, 3)


# ---------------------------------------------------------------------------
# Runner: full-input -> shard over 8 cores -> gather
# ---------------------------------------------------------------------------
_NC_CACHE = {}


def _get_nc():
    if "nc" not in _NC_CACHE:
        _install_drain_patch()
        _NC_CACHE["nc"] = build()
    return _NC_CACHE["nc"]


def run_sharded(inputs, trace=False, trace_kwargs=None):
    """inputs: full-size arrays keyed as in reference.setup_inputs()."""
    from concourse.bass_utils import run_bass_kernel_spmd

    guide = np.asarray(inputs["guide"], dtype=np.float32)
    hidden = np.asarray(inputs["hidden_rep"], dtype=np.float32)
    B = hidden.shape[0]
    assert B == 8 and hidden.shape[1:] == (C, H, W)
    Wq = np.asarray(inputs["Wq"], dtype=np.float32)
    Wk = np.asarray(inputs["Wk"], dtype=np.float32)
    Wv = np.asarray(inputs["Wv"], dtype=np.float32)
    bq = np.asarray(inputs["bq"], dtype=np.float32).reshape(1, C)
    bk = np.asarray(inputs["bk"], dtype=np.float32).reshape(1, C)
    bv = np.asarray(inputs["bv"], dtype=np.float32).reshape(1, C)

    in_maps = []
    for b in range(B):
        in_maps.append({
            "hidden": np.ascontiguousarray(hidden[b].reshape(C, HW)),
            "guide": np.ascontiguousarray(guide[b:b + 1]),
            "Wq": Wq, "Wk": Wk, "Wv": Wv,
            "bq": bq, "bk": bk, "bv": bv,
        })

    nc = _get_nc()
    kw = {}
    if trace:
        kw["trace"] = True
        kw.update(trace_kwargs or {})
    res = run_bass_kernel_spmd(nc, in_maps, list(range(B)), **kw)
    out = np.stack([res.results[b]["out"].reshape(C, H, W) for b in range(B)])
    return out.astype(np.float32), res


def kernel(**inputs):
    out, _ = run_sharded(inputs)
    return out


# revision 24
# speedup vs baseline: 1.0013x; 1.0013x over previous
"""Trainium2 Bass kernel for nn_MultiHeadCrossAttention (B,C,H,W = 8,512,64,64).

Self-contained: builds one single-core Bass/Tile program and runs it SPMD on
8 NeuronCores (data-parallel, one batch element per core).

v2: fully pipelined by 512-token chunk.  x is DMA'd in (t8, i) chunks on the
sync queue (weights go on the scalar queue so they never stall the x stream);
G folds, V projection, SK, scores/exp all chase the DMA stream.  V is stored
dd-contiguous so the AV stage's LDWEIGHTS are contiguous, and AV matmuls are
interleaved over (r2, n2) PE quadrants so 64x64 matmuls overlap in the array.
The PE instruction stream is kept dense so the HAM clock gate stays at 2.4GHz.
"""
import sys

for _p in ("/opt/trn_rl_repo", "/root/.axon_site/_ro/trn_rl_repo"):
    if _p not in sys.path:
        sys.path.append(_p)

import numpy as np


# ---------------------------------------------------------------------------
# Workaround: this walrus build caps sync-waits per CTRL instruction; the
# TileContext exit drain accumulates one wait per active processor and blows
# the cap.  Pre-absorb each wait on its own SP nop before the drain.
# ---------------------------------------------------------------------------
def _install_drain_patch():
    import concourse.tile as tile
    from concourse.vector_clock import ScopedClock

    if getattr(tile.TileContext, "_drain_patch_installed", False):
        return

    def _patched(self, tick_clock, wait_clock):
        nc = self.nc
        gc = tick_clock.global_clock
        scoped = gc if hasattr(gc, "items") else ScopedClock({None: gc})
        for scope, clock in scoped.items():
            for i in range(32):
                try:
                    t = clock.peek_next(i) - 1
                except Exception:
                    break
                if t > 0:
                    nop = nc.sync.nop(nofuse=True, hint="drain_split")
                    sc = ScopedClock()
                    sc.require_at_least(scope, i, t)
                    wait_clock.add_sem_waits(nop.ins, sc)
        nc.sync.drain()  # nops above absorbed every wait; SP is in-order

        nc.all_engine_barrier()
        assert self.sems is not None
        popped = nc._tile_sem_poison_stack.pop()
        assert popped is self._sem_poison
        nc.clear_and_free_semaphores(list(self.sems.allocated().values()))
        nc.all_engine_barrier()

    tile.TileContext._drain_and_barrier = _patched
    tile.TileContext._drain_patch_installed = True


import concourse.bass as bass
import concourse.tile as tile
from concourse import mybir

F32 = mybir.dt.float32
F32R = mybir.dt.float32r
BF16 = mybir.dt.bfloat16
AF = mybir.ActivationFunctionType
ALU = mybir.AluOpType
AX = mybir.AxisListType

C, HW, NH, D, H, W = 512, 4096, 8, 64, 64, 64
NB = 4  # 128-partition blocks of C


def _split_excess_waits(nc, cap=2):
    """This walrus build caps sync-waits per ISA instruction.  Move excess
    waits onto same-engine NoOps inserted just before the instruction
    (same engine => executes immediately before it; semantically identical)."""
    k = 0
    for fn in nc.m.functions:
        for blk in fn.blocks:
            out, changed = [], False
            for inst in blk.instructions:
                si = inst.sync_info
                icap = 1
                if si is not None and len(si.on_wait) > icap:
                    waits = list(si.on_wait)
                    excess, keep = waits[:-icap], waits[-icap:]
                    while excess:
                        chunk, excess = excess[:1], excess[1:]
                        k += 1
                        nop = mybir.InstNoOp(
                            name=f"I-waitsplit-{k}", engine=inst.engine
                        )
                        nop.sync_info = mybir.SyncInfo(
                            on_wait=chunk, on_update=[]
                        )
                        nc.register_instruction(nop)
                        out.append(nop)
                    inst.sync_info = mybir.SyncInfo(
                        on_wait=keep, on_update=list(si.on_update)
                    )
                    changed = True
                out.append(inst)
            if changed:
                blk.instructions = out
    return k


def build():
    nc = bass.Bass("TRN2", target_bir_lowering=False, debug=False, num_devices=1)

    hidden = nc.dram_tensor("hidden", [C, HW], F32R, kind="ExternalInput")
    guide = nc.dram_tensor("guide", [1, C], F32, kind="ExternalInput")
    Wq = nc.dram_tensor("Wq", [C, C], F32, kind="ExternalInput")
    Wk = nc.dram_tensor("Wk", [C, C], F32, kind="ExternalInput")
    Wv = nc.dram_tensor("Wv", [C, C], F32, kind="ExternalInput")
    bq = nc.dram_tensor("bq", [1, C], F32, kind="ExternalInput")
    bk = nc.dram_tensor("bk", [1, C], F32, kind="ExternalInput")
    bv = nc.dram_tensor("bv", [1, C], F32, kind="ExternalInput")
    out = nc.dram_tensor("out", [C, HW], F32, kind="ExternalOutput")

    with tile.TileContext(nc) as tc:
        _body(nc, tc, hidden, guide, Wq, Wk, Wv, bq, bk, bv, out)
    _split_excess_waits(nc)
    return nc


def _body(nc, tc, hidden, guide, Wq, Wk, Wv, bq, bk, bv, out):
    import contextlib

    ctx = contextlib.ExitStack()
    with ctx:
        P = ctx.enter_context(tc.tile_pool(name="persist", bufs=1))
        WN = ctx.enter_context(tc.tile_pool(name="wnat", bufs=4))
        GF = ctx.enter_context(tc.tile_pool(name="gfold", bufs=1))
        SC = ctx.enter_context(tc.tile_pool(name="scpool", bufs=2))
        SS = ctx.enter_context(tc.tile_pool(name="sksb", bufs=1))
        WQ = ctx.enter_context(tc.tile_pool(name="wtq", bufs=1))
        AS = ctx.enter_context(tc.tile_pool(name="attsb", bufs=2))
        PS = ctx.enter_context(tc.tile_pool(name="ps", bufs=2, space="PSUM"))
        PZ = ctx.enter_context(tc.tile_pool(name="pz", bufs=2, space="PSUM"))
        PA = ctx.enter_context(tc.tile_pool(name="pa", bufs=4, space="PSUM"))

        # ---------------- constants ----------------
        ident = P.tile([128, 128], F32, tag="ident")
        from concourse.masks import make_identity

        make_identity(nc, ident[:])
        one1 = P.tile([1, 1], F32, tag="one1")
        nc.vector.memset(one1[:], 1.0)
        # ones column with entries at bases 0/32/64/96 (identity for row
        # transposes of the packed guide/bias rows)
        one_pk = P.tile([97, 1], F32, tag="one_pk")
        nc.vector.memset(one_pk[:], 1.0)
        identpk = P.tile([66, 2], F32, tag="identpk")
        nc.vector.memset(identpk[0:2, :], 0.0)
        # half-indicator rows for QV2 broadcast (K=1 matmuls)
        ones1a = P.tile([1, 128], F32, tag="ones1a")
        ones1b = P.tile([1, 128], F32, tag="ones1b")
        nc.vector.memset(ones1a[:], 0.0)
        nc.vector.memset(ones1a[:, 0:64], 1.0)
        nc.vector.memset(ones1b[:], 0.0)
        nc.vector.memset(ones1b[:, 64:128], 1.0)
        # column indicator [128,2] (lhsT for Z row sums over each 64-half)
        i2colsT = P.tile([128, 2], BF16, tag="i2colsT")
        nc.vector.memset(i2colsT[:], 0.0)
        nc.vector.memset(i2colsT[0:64, 0:1], 1.0)
        nc.vector.memset(i2colsT[64:128, 1:2], 1.0)
        # [2,128] row-indicator blocks at partition bases 32*j2 (lhsT for the
        # partition-broadcast of zinv rows; base must match the zn rhs base)
        i2colsF = P.tile([128, 2], F32, tag="i2colsF")
        nc.vector.memset(i2colsF[:], 0.0)
        nc.vector.memset(i2colsF[0:64, 0:1], 1.0)
        nc.vector.memset(i2colsF[64:128, 1:2], 1.0)
        i2rowsB = P.tile([98, 128], BF16, tag="i2rowsB")
        i2rp = PZ.tile([2, 128], F32, tag="small")
        nc.tensor.transpose(out=i2rp[:], in_=i2colsF[:], identity=ident[:])
        for j2 in range(NB):
            nc.vector.tensor_copy(i2rowsB[32 * j2:32 * j2 + 2, :], i2rp[:])

        # ------- input DMAs: weights on scalar queue, x on sync queue -------
        wnat = {}
        for nm, dr in (("Wv", Wv), ("Wk", Wk), ("Wq", Wq)):
            nat = []
            for cb in range(NB):
                t = WN.tile([128, C], F32, tag="wtrans")
                nc.scalar.dma_start(out=t, in_=dr.ap()[cb * 128:(cb + 1) * 128, :])
                nat.append(t)
            wnat[nm] = nat
        # guide + biases packed at partition bases 0/32/64/96 of one tile
        rowpk = P.tile([97, C], F32, tag="rowpk")
        nc.scalar.dma_start(out=rowpk[0:1, :], in_=guide.ap())
        nc.scalar.dma_start(out=rowpk[32:33, :], in_=bq.ap())
        nc.scalar.dma_start(out=rowpk[64:65, :], in_=bk.ap())
        nc.scalar.dma_start(out=rowpk[65:66, :], in_=bv.ap())
        nc.sync.dma_start(out=identpk[64:66, :], in_=ident[0:2, 0:2])
        guide_sb = rowpk[0:1, :]
        brow = {"bq": rowpk[32:33, :], "bkv": rowpk[64:66, :]}
        # x tiles, DMA'd per (t8, i) 512-token chunk so compute can chase
        x = [P.tile([128, HW], F32R, tag=f"x{j}", name=f"x{j}") for j in range(NB)]
        for t8 in range(8):
            for i in range(NB):
                nc.sync.dma_start(
                    out=x[i][:, t8 * 512:(t8 + 1) * 512],
                    in_=hidden.ap()[i * 128:(i + 1) * 128,
                                    t8 * 512:(t8 + 1) * 512],
                )

        # ---------------- weight transposes (PE) ----------------
        # wT[j] [128,512]: wT[j][p, c_out] = W[c_out, 128j+p]
        def transpose_weight(nm, j, dst_pool, dtype, copy_eng, tag=None):
            pt = PS.tile([128, C], F32, tag="mm512")
            for cb in range(NB):
                nc.tensor.transpose(
                    out=pt[:, cb * 128:(cb + 1) * 128],
                    in_=wnat[nm][cb][:, j * 128:(j + 1) * 128],
                    identity=ident[:],
                )
            sb = dst_pool.tile([128, C], dtype, tag=tag or f"{nm}T{j}",
                               name=f"{nm}T{j}")
            if copy_eng is nc.scalar:
                copy_eng.copy(sb[:], pt[:])
            else:
                copy_eng.tensor_copy(sb[:], pt[:])
            return sb

        wTv = [transpose_weight("Wv", j, P, F32R, nc.vector) for j in range(NB)]  # noqa
        wTk = [transpose_weight("Wk", j, P, F32, nc.scalar) for j in range(NB)]

        # ---------------- bias columns (PE transpose of rows) ----------------
        bkcol, bvcol = [], []
        for j in range(NB):
            pt = PZ.tile([128, 2], F32, tag="small")
            nc.tensor.transpose(
                out=pt[:], in_=brow["bkv"][:, j * 128:(j + 1) * 128],
                identity=identpk[64:66, :], tile_position=(64, 0),
            )
            kc = P.tile([128, 1], F32, tag=f"bk64c{j}")
            vc = P.tile([128, 1], F32, tag=f"bvc{j}")
            nc.scalar.mul(kc[:], pt[:, 0:1], 64.0)  # K bias enters via 64-tok sum
            nc.scalar.copy(vc[:], pt[:, 1:2])
            bkcol.append(kc)
            bvcol.append(vc)
        gcol = []
        for j in range(NB):
            pt = PZ.tile([128, 1], F32, tag="small")
            nc.tensor.transpose(
                out=pt[:], in_=guide_sb[:, j * 128:(j + 1) * 128],
                identity=one1[:],
            )
            t = P.tile([128, 1], F32, tag=f"gcol{j}")
            nc.vector.tensor_copy(t[:], pt[:])
            gcol.append(t)

        # ---------------- q path (per-block WqT, consumed immediately) -----
        qp = PZ.tile([1, C], F32, tag="small")
        for i in range(NB):
            wTq_i = transpose_weight("Wq", i, WQ, F32, nc.scalar, tag="wtq")
            nc.tensor.matmul(
                out=qp[:], lhsT=gcol[i][:], rhs=wTq_i[:],
                start=(i == 0), stop=False,
            )
        nc.tensor.matmul(
            out=qp[:], lhsT=one_pk[32:33, 0:1], rhs=brow["bq"],
            start=False, stop=True, tile_position=(32, 0),
        )
        q_sc = P.tile([1, C], F32, tag="q_sc")
        nc.scalar.copy(q_sc[:], qp[:])
        qv2 = []
        for j in range(NB):
            pt = PZ.tile([128, 64], F32, tag="small")
            nc.tensor.matmul(
                out=pt[:], lhsT=ones1a[:],
                rhs=q_sc[:, (2 * j) * 64:(2 * j + 1) * 64],
                start=True, stop=False,
            )
            nc.tensor.matmul(
                out=pt[:], lhsT=ones1b[:],
                rhs=q_sc[:, (2 * j + 1) * 64:(2 * j + 2) * 64],
                start=False, stop=True,
            )
            t = P.tile([128, 64], F32, tag=f"qv2_{j}")
            nc.scalar.mul(t[:], pt[:], 0.125)  # fold scale = 1/sqrt(d)
            qv2.append(t)

        # ---------------- persistent state for the chunk pipeline ----------
        g = [P.tile([128, 64], F32, tag=f"g{j}", name=f"g{j}") for j in range(NB)]
        vt = [P.tile([128, HW], BF16, tag=f"vt{j}", name=f"vt{j}") for j in range(NB)]
        e = [P.tile([128, HW], BF16, tag=f"e{j2}", name=f"e{j2}") for j2 in range(NB)]
        skT = [P.tile([128, 64], F32, tag=f"skT{j2}", name=f"skT{j2}") for j2 in range(NB)]
        # Z staging packed at partition bases 32*j2 (rows 32*j2 + r2); the
        # unused rows make the free-range shared so SBUF cost is one tile.
        zsbig = P.tile([98, HW], BF16, tag="zsbig")
        znbig = P.tile([98, HW], BF16, tag="znbig")
        # zr[j2] [128, 64]: p = r2*64 + (n*8 + g'), f = ww
        zr = [P.tile([128, 64], BF16, tag=f"zr{j2}", name=f"zr{j2}") for j2 in range(NB)]
        zinv = [P.tile([128, 64], BF16, tag=f"zi{j2}", name=f"zinv{j2}") for j2 in range(NB)]

        def g_chunk(i, t8):
            # fold dd 64->32 on gpsimd, then DVE strided reduce over dd(32);
            # two ops per chunk so G tracks the x DMA stream with ~1us lag
            xc = x[i][:, t8 * 512:(t8 + 1) * 512].bitcast(F32)
            fa = GF.tile([128, 256], F32, tag="fa", name=f"fa{i}_{t8}")
            nc.gpsimd.tensor_add(fa[:], xc[:, 0:256], xc[:, 256:512])
            nc.vector.tensor_reduce(
                out=g[i][:, t8 * 8:(t8 + 1) * 8],
                in_=fa[:].rearrange("p (dd gg) -> p gg dd", gg=8),
                axis=AX.X, op=ALU.add,
            )

        def vproj_chunk(t8, j, eng):
            pt = PS.tile([128, 512], F32, tag="mm512")
            for i in range(NB):
                nc.tensor.matmul(
                    out=pt[:],
                    lhsT=wTv[i][:, j * 128:(j + 1) * 128],
                    rhs=x[i][:, t8 * 512:(t8 + 1) * 512],
                    start=(i == 0), stop=(i == NB - 1),
                )
            # permute free axis (dd*8+g') -> (g'*64+dd) so AV lhsT is contiguous
            dst = vt[j][:, t8 * 512:(t8 + 1) * 512].rearrange(
                "p (gg dd) -> p gg dd", gg=8)
            src = pt[:].rearrange("p (dd gg) -> p gg dd", gg=8)
            if eng % 2 == 0:
                nc.scalar.activation(dst, src, AF.Identity, bias=bvcol[j][:])
            else:
                nc.vector.tensor_scalar_add(dst, src, bvcol[j][:])

        def sk_half(h):
            # SKT2h [32, 512] = (G-half)^T contracted with WkT, then PE
            # transpose per j2-block into skT[j2][:, h*32:(h+1)*32]
            spt = PZ.tile([32, 512], F32, tag="small")
            for i in range(NB):
                nc.tensor.matmul(
                    out=spt[:], lhsT=g[i][:, h * 32:(h + 1) * 32],
                    rhs=wTk[i][:], start=(i == 0), stop=(i == NB - 1),
                )
            ssb = SS.tile([32, 512], F32, tag="sksb")
            nc.scalar.copy(ssb[:], spt[:])
            for j2 in range(NB):
                tpt = PZ.tile([128, 32], F32, tag="small")
                nc.tensor.transpose(
                    out=tpt[:], in_=ssb[:, j2 * 128:(j2 + 1) * 128],
                    identity=ident[0:32, 0:32],
                )
                nc.vector.tensor_scalar_add(
                    skT[j2][:, h * 32:(h + 1) * 32], tpt[:], bkcol[j2][:])

        def scores_exp_chunk(j2, n):
            sc_t = SC.tile([128, 512], F32, tag="sc")
            in0 = (qv2[j2][:].rearrange("p (a q) -> p a q", a=1)
                   .broadcast_to((128, 8, 64)))
            in1 = skT[j2][:, n * 8:(n + 1) * 8].broadcast_to((128, 8, 64))
            o3 = sc_t[:].rearrange("p (s q) -> p s q", q=64)
            if n < 4:
                nc.gpsimd.tensor_mul(o3, in0, in1)
            else:
                nc.vector.tensor_mul(o3, in0, in1)
            nc.scalar.activation(
                e[j2][:, n * 512:(n + 1) * 512], sc_t[:], AF.Exp)

        # ---------------- chunk pipeline ----------------
        def z_group(n):
            zpb = PZ.tile([98, 512], F32, tag="small", name=f"zpb{n}")
            nc.vector.memset(zpb[:], 0.0)
            for j2 in range(NB):
                nc.tensor.matmul(
                    out=zpb[32 * j2:32 * j2 + 2, :], lhsT=i2colsT[:],
                    rhs=e[j2][:, n * 512:(n + 1) * 512],
                    start=True, stop=True, tile_position=(0, 32 * j2),
                )
            with nc.allow_low_precision(reason="bf16 Z staging"):
                if n % 2 == 0:
                    nc.vector.tensor_copy(zsbig[:, n * 512:(n + 1) * 512], zpb[:])
                else:
                    nc.scalar.copy(zsbig[:, n * 512:(n + 1) * 512], zpb[:])

        def z_tail_half(h):
            # gather Z rows for this n-half into 32-aligned partition blocks
            # of zr, reciprocal, scatter back to znbig.  DMAs ride the vector
            # ring so they never queue behind 1MB output DMAs.
            for j2 in range(NB):
                for r2 in range(2):
                    nc.sync.dma_start(
                        out=zr[j2][r2 * 64 + h * 32:r2 * 64 + (h + 1) * 32, :],
                        in_=zsbig[32 * j2 + r2:32 * j2 + r2 + 1,
                                  h * 2048:(h + 1) * 2048].rearrange(
                            "p (a w) -> p a w", w=64),
                    )
                with nc.allow_low_precision(reason="bf16 softmax normalizer"):
                    for r2 in range(2):
                        sl = slice(r2 * 64 + h * 32, r2 * 64 + (h + 1) * 32)
                        nc.vector.reciprocal(zinv[j2][sl, :], zr[j2][sl, :])
                for r2 in range(2):
                    nc.sync.dma_start(
                        out=znbig[32 * j2 + r2:32 * j2 + r2 + 1,
                                  h * 2048:(h + 1) * 2048].rearrange(
                            "p (a w) -> p a w", w=64),
                        in_=zinv[j2][r2 * 64 + h * 32:r2 * 64 + (h + 1) * 32, :],
                    )

        def norm_half(j2, h):
            for n in range(h * 4, (h + 1) * 4):
                zb = PZ.tile([128, 512], F32, tag="small")
                nc.tensor.matmul(
                    out=zb[:], lhsT=i2rowsB[32 * j2:32 * j2 + 2, :],
                    rhs=znbig[32 * j2:32 * j2 + 2, n * 512:(n + 1) * 512],
                    start=True, stop=True, tile_position=(32 * j2, 0),
                )
                es = e[j2][:, n * 512:(n + 1) * 512]
                nc.vector.tensor_mul(es, es, zb[:])

        for t8 in range(8):
            if t8 < 7:
                for i in range(NB):
                    g_chunk(i, t8)
            if t8 == 6:
                for i in range(NB):
                    g_chunk(i, 7)
            for j in range(NB):
                vproj_chunk(t8, j, eng=(t8 * NB + j) % 2)
            if t8 == 3:
                sk_half(0)
                for n in range(4):
                    for j2 in range(NB):
                        scores_exp_chunk(j2, n)
            if t8 == 6:
                sk_half(1)
                for n in range(4, 8):
                    for j2 in range(NB):
                        scores_exp_chunk(j2, n)
            if t8 == 7:
                for n in range(4):
                    z_group(n)

        z_tail_half(0)

        def av_block(j2, jp):
            if True:
                at = [PA.tile([128, 512], F32, tag="att", name=f"at{j2}_{jp}_{k}")
                      for k in range(2)]
                for gg in range(8):
                    for r2, n2 in ((0, 0), (1, 1), (0, 1), (1, 0)):
                        n = 2 * jp + n2
                        sl = slice(n * 512 + gg * 64, n * 512 + (gg + 1) * 64)
                        nc.tensor.matmul(
                            out=at[r2][n2 * 64:(n2 + 1) * 64,
                                       gg * 64:(gg + 1) * 64],
                            lhsT=vt[j2][r2 * 64:(r2 + 1) * 64, sl],
                            rhs=e[j2][r2 * 64:(r2 + 1) * 64, sl],
                            start=True, stop=True,
                        )
                asb = AS.tile([128, 1024], F32, tag="attsb",
                              name=f"asb{j2}_{jp}")
                av = asb[:].rearrange("p (gg rr q) -> p gg rr q", rr=2, q=64)
                for r2 in range(2):
                    srcv = at[r2][:].rearrange("p (gg q) -> p gg q", q=64)
                    if (jp + r2) % 2 == 0:
                        nc.vector.tensor_copy(av[:, :, r2, :], srcv)
                    else:
                        nc.scalar.copy(av[:, :, r2, :], srcv)
                dst = out.ap()[jp * 128:(jp + 1) * 128, :].rearrange(
                    "p (gg rr2 q2) -> p rr2 gg q2", rr2=4, q2=128
                )[:, j2, :, :]
                srcd = asb[:].rearrange("p (gg q2) -> p gg q2", q2=128)
                eng = [nc.sync, nc.scalar][(j2 + jp) % 2]
                eng.dma_start(out=dst, in_=srcd)

        # half 0 (jp 0-1) normalizes and runs right after Vproj; the half-1
        # Z chain is interleaved mid-stream so the PE never idles.
        for j2 in range(NB):
            norm_half(j2, 0)
        av_block(0, 0)
        av_block(0, 1)
        for n in range(4, NH):
            z_group(n)
        av_block(1, 0)
        av_block(1, 1)
        z_tail_half(1)
        av_block(2, 0)
        av_block(2, 1)
        for j2 in range(NB):
            norm_half(j2, 1)
        av_block(3, 0)
        av_block(3, 1)
        for j2 in range(NB):
            av_block(j2, 2)
            av_block(j2, 3)


# ---------------------------------------------------------------------------
# Runner: full-input -> shard over 8 cores -> gather
# ---------------------------------------------------------------------------
_NC_CACHE = {}


def _get_nc():
    if "nc" not in _NC_CACHE:
        _install_drain_patch()
        _NC_CACHE["nc"] = build()
    return _NC_CACHE["nc"]


def run_sharded(inputs, trace=False, trace_kwargs=None):
    """inputs: full-size arrays keyed as in reference.setup_inputs()."""
    from concourse.bass_utils import run_bass_kernel_spmd

    guide = np.asarray(inputs["guide"], dtype=np.float32)
    hidden = np.asarray(inputs["hidden_rep"], dtype=np.float32)
    B = hidden.shape[0]
    assert B == 8 and hidden.shape[1:] == (C, H, W)
    Wq = np.asarray(inputs["Wq"], dtype=np.float32)
    Wk = np.asarray(inputs["Wk"], dtype=np.float32)
    Wv = np.asarray(inputs["Wv"], dtype=np.float32)
    bq = np.asarray(inputs["bq"], dtype=np.float32).reshape(1, C)
    bk = np.asarray(inputs["bk"], dtype=np.float32).reshape(1, C)
    bv = np.asarray(inputs["bv"], dtype=np.float32).reshape(1, C)

    in_maps = []
    for b in range(B):
        in_maps.append({
            "hidden": np.ascontiguousarray(hidden[b].reshape(C, HW)),
            "guide": np.ascontiguousarray(guide[b:b + 1]),
            "Wq": Wq, "Wk": Wk, "Wv": Wv,
            "bq": bq, "bk": bk, "bv": bv,
        })

    nc = _get_nc()
    kw = {}
    if trace:
        kw["trace"] = True
        kw.update(trace_kwargs or {})
    res = run_bass_kernel_spmd(nc, in_maps, list(range(B)), **kw)
    out = np.stack([res.results[b]["out"].reshape(C, H, W) for b in range(B)])
    return out.astype(np.float32), res


def kernel(**inputs):
    out, _ = run_sharded(inputs)
    return out
